# revision 33
# baseline (speedup 1.0000x reference)
"""Trainium2 Bass kernel: BERT self-attention with granularity-gated sparse
penalties, exploiting the data-dependent banded mask.

Math: softmax(S/8 + log(max(pen,1e-32))) == pen*exp(S/8) / sum(pen*exp(S/8)).
pen = res*scope with scope = clip(w_q+2 - |i-j|, 0, 1): a per-query BAND of
half-width w_q+2 = (S-2)^(1-z_q)+2.  ~88% of (q,k) pairs are exactly masked.

Sharding (8 cores): core c -> batch c//4, heads 4*(c%4)..+4 (dims 256*(c%4)).
K/V/Q projections computed only for the core's 256 dims (no redundancy).

Sparsity schedule (host, data-dependent, compiled per input):
 - queries sorted by band-width rank into chunks of CW=256 (index-sorted
   inside); per (chunk, key-tile kt) the active queries form a contiguous
   segment [A,B) after monotone closure (A,B non-decreasing in kt).
 - segments are the UNION over both batches -> identical program structure
   for all 8 cores (SPMD); extra columns self-zero via that batch's pen.
 - scores/exp/pen-mul/PV run only on segment columns (~25% of dense).

Per (pair of heads, group of 1024 queries): kt-loop; scores [128k x W] into
sp PSUM ([h0 bank | h1 bank]); one exp (ACT) per window; one pen-mul (DVE,
broadcast over the 2 heads); PV accumulates V^T@E into pv [65, 1024] PSUM
using per-byte pending-zero semantics (split at B_prev, bank start/stop at
first/last touch).  l = ones column 65 of V; host divides and un-permutes.

Penalties are host-precomputed (0.03% of FLOPs) packed [128, PENW] bf16 in
window order and DMA'd; gate z computed host-side in f64.
"""

import math

import ml_dtypes
import numpy as np

import concourse.bass as bass
import concourse.tile as tile
from concourse import bacc, mybir
from concourse.bass import AP
from concourse.bass_utils import run_bass_kernel_spmd

F32 = mybir.dt.float32
BF16 = mybir.dt.bfloat16
FP16 = mybir.dt.float16
AF = mybir.ActivationFunctionType
OP = mybir.AluOpType

B, S, H = 2, 2048, 1024
NH, HD = 16, 64
NC = 8
KT = 16            # key tiles of 128
CW = 512           # closure chunk width (queries) == one pv PSUM bank
NCH = S // CW      # 4 chunks
VW = HD + 1        # 65: V dims + ones column
GW = 1024          # query group width (pv tile)
WCAP = 512         # max window cols per head (one PSUM bank)
LAG = 3
LN_BASE = float(np.log(np.float32(S - 2)))


# ---------------------------------------------------------------- planning

def _gate_z(hidden, Wg, bg):
    Wg_f = np.asarray(Wg, np.float64).reshape(H)
    bg_f = float(np.asarray(bg, np.float64).reshape(()))
    pre = np.asarray(hidden, np.float64).reshape(B * S, H) @ Wg_f + bg_f
    return (1.0 / (1.0 + np.exp(-pre))).reshape(B, S)


def _make_plan(z):
    """Common (union over batches) sparse schedule + per-batch permutations."""
    idx = np.arange(S)
    t = np.exp((1.0 - z) * LN_BASE) + 2.0   # band half-width per query
    perms = []
    segs_b = np.full((B, NCH, KT, 2), -1, np.int64)
    for b in range(B):
        rank = np.argsort(np.argsort(t[b], kind="stable"), kind="stable")
        perm = np.lexsort((idx, rank // CW))
        perms.append(perm)
        lo = np.clip(np.floor(idx - t[b]) - 1, 0, S - 1).astype(np.int64)
        hi = np.clip(np.ceil(idx + t[b]) + 1, 0, S - 1).astype(np.int64)
        ktlo = lo[perm] // 128
        kthi = hi[perm] // 128
        for c in range(NCH):
            sl = slice(c * CW, (c + 1) * CW)
            kl = np.minimum.accumulate(ktlo[sl][::-1])[::-1]
            kh = np.maximum.accumulate(kthi[sl])
            for kt in range(KT):
                act = np.nonzero((kl <= kt) & (kh >= kt))[0]
                if len(act):
                    segs_b[b, c, kt] = (c * CW + act.min(), c * CW + act.max() + 1)

    # union across batches, then monotone closure (A suffix-min, B prefix-max)
    chunk_segs = []   # per chunk: list of (kt, A, B, Bprev)
    for c in range(NCH):
        A = np.full(KT, 1 << 30, np.int64)
        Bn = np.full(KT, -1, np.int64)
        for b in range(B):
            for kt in range(KT):
                a, bb = segs_b[b, c, kt]
                if a >= 0:
                    A[kt] = min(A[kt], a)
                    Bn[kt] = max(Bn[kt], bb)
        ne = np.nonzero(Bn >= 0)[0]
        k0, k1 = int(ne.min()), int(ne.max())
        for kt in range(k1 - 1, k0 - 1, -1):
            A[kt] = min(A[kt], A[kt + 1])
        for kt in range(k0 + 1, k1 + 1):
            Bn[kt] = max(Bn[kt], Bn[kt - 1])
        segs = []
        for kt in range(k0, k1 + 1):
            if A[kt] < Bn[kt]:
                bprev = int(Bn[kt - 1]) if kt > k0 else int(A[kt])
                bprev = min(max(bprev, int(A[kt])), int(Bn[kt]))
                segs.append((kt, int(A[kt]), int(Bn[kt]), bprev))
        chunk_segs.append(segs)

    # windows per group: kt-ordered segments packed across kt up to WCAP
    # cols per head (one sp PSUM bank); each seg carries its own kt.
    sched = [[] for _ in range(2)]      # sched[g] = [win], win = dict
    for g in range(2):
        segs = []
        for kt in range(KT):
            for c in range(2 * g, 2 * g + 2):
                for (skt, a, bb, bp) in chunk_segs[c]:
                    if skt == kt:
                        segs.append({"kt": kt, "c": c, "A": a, "B": bb})
        cur = []
        cw = 0
        for sg in segs:
            w = sg["B"] - sg["A"]
            if cur and cw + w > WCAP:
                sched[g].append({"segs": cur, "sw": cw})
                cur, cw = [], 0
            sg["off"] = cw
            cur.append(sg)
            cw += w
        if cur:
            sched[g].append({"segs": cur, "sw": cw})

    # pen offsets, interleaving groups by window start-kt so the pen DMA
    # stream is consumed roughly front-to-back
    off = 0
    for win in sorted(
        [w for g in range(2) for w in sched[g]], key=lambda w: w["segs"][0]["kt"]
    ):
        win["pen_off"] = off
        off += win["sw"]
    penw = off + (-off) % 8

    # PSUM bank start/stop flags: one PV matmul per (seg, head); HW per-byte
    # pending-zero handles mixed fresh/accumulate ranges.  start=True (bank
    # zeroing) only on the chunk-bank's first touch, stop on its last.
    for g in range(2):
        for c in range(2 * g, 2 * g + 2):
            seq = []
            for win in sched[g]:
                for sg in win["segs"]:
                    if sg["c"] == c:
                        sg["st"] = False
                        sg["sp"] = False
                        seq.append(sg)
            seq[0]["st"] = True
            seq[-1]["sp"] = True

    sig = (penw, tuple(
        (g, tuple((sg["kt"], sg["c"], sg["A"], sg["B"], sg["st"], sg["sp"])
                  for sg in win["segs"]))
        for g in range(2) for win in sched[g]
    ))
    return {"perms": perms, "sched": sched, "penw": penw, "t": t, "sig": sig}


# ---------------------------------------------------------------- device

def _pm_view(ap_1d, n_free):
    return AP(tensor=ap_1d.tensor, offset=ap_1d.offset, ap=[[1, 128], [128, n_free]])


def _wview(w2d, dt, width):
    return AP(
        tensor=w2d.tensor,
        offset=w2d.offset + dt * 8 * 128 * width,
        ap=[[width, 128], [128 * width, 8], [1, width]],
    )


def build_nc(plan):
    nc = bacc.Bacc("TRN2", target_bir_lowering=False, debug=False)
    penw = plan["penw"]
    hT = nc.dram_tensor("hT", [H, S], FP16, kind="ExternalInput").ap()
    hTq = nc.dram_tensor("hTq", [H, S], FP16, kind="ExternalInput").ap()
    Wk = nc.dram_tensor("Wk", [2 * 8 * 128, 128], FP16, kind="ExternalInput").ap()
    Wq = nc.dram_tensor("Wq", [2 * 8 * 128, 128], FP16, kind="ExternalInput").ap()
    Wv = nc.dram_tensor("Wv", [8 * 128, 256], FP16, kind="ExternalInput").ap()
    bkq = nc.dram_tensor("bkq", [4 * 128], F32, kind="ExternalInput").ap()
    bvp = nc.dram_tensor("bvp", [VW, 4], F32, kind="ExternalInput").ap()
    pen = nc.dram_tensor("pen", [128, penw], BF16, kind="ExternalInput").ap()
    out = nc.dram_tensor("out", [4 * VW, S], BF16, kind="ExternalOutput").ap()

    with tile.TileContext(nc) as tc:
        _body(tc, nc, plan, hT, hTq, Wk, Wq, Wv, bkq, bvp, pen, out)
    nc.compile()
    return nc


def _body(tc, nc, plan, hT, hTq, Wk, Wq, Wv, bkq, bvp, pen, out):
    import contextlib

    penw = plan["penw"]
    sched = plan["sched"]
    ctx = contextlib.ExitStack()
    with ctx:
        big = ctx.enter_context(tc.tile_pool(name="big", bufs=1))
        wpool = ctx.enter_context(tc.tile_pool(name="wp", bufs=1))
        epool = ctx.enter_context(tc.tile_pool(name="ep", bufs=7))
        cpool = ctx.enter_context(tc.tile_pool(name="cp", bufs=4))

        hT_ts = [big.tile([128, S], FP16, name=f"hT{i}", tag=f"hT{i}") for i in range(8)]
        hTq_ts = [
            big.tile([128, S], FP16, name=f"hTq{i}", tag=f"hTq{i}") for i in range(8)
        ]
        ktT = big.tile([128, 2 * S], FP16)
        qT = big.tile([128, 2 * S], FP16)
        v_sb = big.tile([128, KT * 4 * VW], BF16)
        pen_sb = big.tile([128, penw], BF16)
        bkq_sb = big.tile([128, 4], F32)
        bvp_sb = big.tile([VW, 4], F32)

        # --- DMA plan (measured queue rates: sync ~155GB/s, gpsimd ~109,
        # scalar ~23): weights first on sync, hidden interleaved over the
        # two fast queues (hT before hTq so K/V chase immediately), pen
        # split into 4 kt-progressive chunks.
        wkb = wpool.tile([128, 2, 8, 128], FP16, name="wkb")
        wqb = wpool.tile([128, 2, 8, 128], FP16, name="wqb")
        wvb = wpool.tile([128, 8, 256], FP16, name="wvb")
        ones_view = v_sb[:, :].rearrange("p (k c) -> p k c", c=VW)[:, :, HD : HD + 1]
        nc.gpsimd.memset(ones_view, 1.0)

        pq = penw // 2
        for dt in range(2):
            nc.sync.dma_start(wkb[:, dt, :, :], _wview(Wk, dt, 128))
        nc.sync.dma_start(wvb[:, :, :], _wview(Wv, 0, 256))
        for ht in range(8):
            nc.sync.dma_start(hT_ts[ht][:, :], hT[ht * 128 : ht * 128 + 128, :])
        for ht in range(6, 8):
            nc.sync.dma_start(hTq_ts[ht][:, :], hTq[ht * 128 : ht * 128 + 128, :])
        nc.sync.dma_start(pen_sb[:, 0:pq], pen[:, 0:pq])
        for dt in range(2):
            nc.scalar.dma_start(wqb[:, dt, :, :], _wview(Wq, dt, 128))
        for ht in range(6):
            nc.scalar.dma_start(hTq_ts[ht][:, :], hTq[ht * 128 : ht * 128 + 128, :])
        nc.scalar.dma_start(pen_sb[:, pq:], pen[:, pq:])
        nc.gpsimd.dma_start(bkq_sb[:, :], _pm_view(bkq, 4))
        nc.gpsimd.dma_start(bvp_sb[:, :], bvp[:, :])

        # shared projection-unit bodies (psum arg: [128, >=512] f32 tile)
        def k_unit(ps, dt, tt, dve=False):
            for ht in range(8):
                nc.tensor.matmul(
                    ps[:, 0:512],
                    wkb[:, dt, ht, :],
                    hT_ts[ht][:, tt * 512 : (tt + 1) * 512],
                    start=(ht == 0),
                    stop=(ht == 7),
                )
            dst = ktT[:, dt * S + tt * 512 : dt * S + (tt + 1) * 512]
            if dve:
                nc.vector.tensor_scalar(dst, ps[:, 0:512], bkq_sb[:, dt : dt + 1], None, OP.add)
            else:
                nc.scalar.activation(dst, ps[:, 0:512], AF.Identity, bias=bkq_sb[:, dt : dt + 1])

        def q_unit(ps, dt, tt, dve=False):
            for ht in range(8):
                nc.tensor.matmul(
                    ps[:, 0:512],
                    wqb[:, dt, ht, :],
                    hTq_ts[ht][:, tt * 512 : (tt + 1) * 512],
                    start=(ht == 0),
                    stop=(ht == 7),
                )
            dst = qT[:, dt * S + tt * 512 : dt * S + (tt + 1) * 512]
            if dve:
                nc.vector.tensor_scalar(dst, ps[:, 0:512], bkq_sb[:, 2 + dt : 3 + dt], None, OP.add)
            else:
                nc.scalar.activation(dst, ps[:, 0:512], AF.Identity, bias=bkq_sb[:, 2 + dt : 3 + dt])

        def v_unit(ps, tv):
            for ht in range(8):
                nc.tensor.matmul(
                    ps[:, 0:256],
                    hT_ts[ht][:, tv * 128 : tv * 128 + 128],
                    wvb[:, ht, :],
                    start=(ht == 0),
                    stop=(ht == 7),
                )
            base = tv * 4 * VW
            dst = v_sb[:, base : base + 4 * VW].rearrange(
                "p (h c) -> p h c", c=VW
            )[:, :, 0:HD]
            src = ps[:, 0:256].rearrange("p (h c) -> p h c", c=HD)
            nc.vector.tensor_scalar(dst, src, 0.0, None, OP.add)

        # --- prefix: K-dt0 (chases hT DMA), V kt 0-3, Q-dt0 (needs hTq)
        with tc.tile_pool(name="pp", bufs=1, space="PSUM") as pp:
            kps = [pp.tile([128, 512], F32, tag=f"a{i}", name=f"a{i}") for i in range(4)]
            qps = [pp.tile([128, 512], F32, tag=f"b{i}", name=f"b{i}") for i in range(4)]
            for ht in range(8):
                for tt in range(4):
                    nc.tensor.matmul(
                        kps[tt][:, :],
                        wkb[:, 0, ht, :],
                        hT_ts[ht][:, tt * 512 : (tt + 1) * 512],
                        start=(ht == 0),
                        stop=(ht == 7),
                    )
            for tt in range(4):
                nc.scalar.activation(
                    ktT[:, tt * 512 : (tt + 1) * 512],
                    kps[tt][:, :],
                    AF.Identity,
                    bias=bkq_sb[:, 0:1],
                )
            for tv in range(4):
                ps = pp.tile([128, 512], F32, tag=f"a{tv}", name=f"av{tv}")
                v_unit(ps, tv)
            for ht in range(8):
                for tt in range(4):
                    nc.tensor.matmul(
                        qps[tt][:, :],
                        wqb[:, 0, ht, :],
                        hTq_ts[ht][:, tt * 512 : (tt + 1) * 512],
                        start=(ht == 0),
                        stop=(ht == 7),
                    )
            for tt in range(4):
                nc.scalar.activation(
                    qT[:, tt * 512 : (tt + 1) * 512],
                    qps[tt][:, :],
                    AF.Identity,
                    bias=bkq_sb[:, 2:3],
                )

        # --- attention: pairs sequential; per (pair, group) a kt loop.
        # Remaining projections (V 4-15, K-dt1, Q-dt1) run as FILLER units
        # inside pair-0's window rotation (same sp PSUM pool) so the PE
        # stream never drains and stays at max p-state.
        fillers = [lambda ps, tv=tv: v_unit(ps, tv) for tv in range(4, KT)]
        fillers += [lambda ps, tt=tt: k_unit(ps, 1, tt, dve=True) for tt in range(4)]
        fillers += [lambda ps, tt=tt: q_unit(ps, 1, tt, dve=True) for tt in range(4)]
        fillers = fillers[::-1]          # pop() from the front

        with (
            tc.tile_pool(name="sp", bufs=2, space="PSUM") as spp,
            tc.tile_pool(name="pvp", bufs=1, space="PSUM") as pvp,
        ):
            for pair in range(2):
                for g in range(2):
                    pv = [
                        pvp.tile([VW, GW], F32, tag=f"pv{h}", name=f"pv{h}")
                        for h in range(2)
                    ]

                    def emit_pv(win, e, pair=pair, g=g, pv=pv):
                        sw = win["sw"]
                        for sg in win["segs"]:
                            kt = sg["kt"]
                            for h in range(2):
                                lhs = v_sb[
                                    :,
                                    kt * 4 * VW + (2 * pair + h) * VW :
                                    kt * 4 * VW + (2 * pair + h) * VW + VW,
                                ]
                                eoff = h * sw + sg["off"]
                                nc.tensor.matmul(
                                    pv[h][:, sg["A"] - GW * g : sg["B"] - GW * g],
                                    lhs,
                                    e[:, eoff : eoff + (sg["B"] - sg["A"])],
                                    start=sg["st"],
                                    stop=sg["sp"],
                                    skip_group_check=True,
                                )

                    wq_q = []
                    for win in sched[g]:
                        sw = win["sw"]
                        sp = spp.tile([128, 1024], F32, tag="sp")
                        nseg = len(win["segs"])
                        for si, sg in enumerate(win["segs"]):
                            a, bb, kt = sg["A"], sg["B"], sg["kt"]
                            for h in range(2):
                                nc.tensor.matmul(
                                    sp[:, h * 512 + sg["off"] : h * 512 + sg["off"] + (bb - a)],
                                    ktT[
                                        h * 64 : h * 64 + 64,
                                        pair * S + kt * 128 : pair * S + kt * 128 + 128,
                                    ],
                                    qT[h * 64 : h * 64 + 64, pair * S + a : pair * S + bb],
                                    start=(si == 0),
                                    stop=(si == nseg - 1),
                                    tile_position=(h * 64, 0),
                                )
                        e = epool.tile([128, 1024], BF16, tag="e")
                        e_view = e[:, 0 : 2 * sw].rearrange("p (r n) -> p r n", r=2)
                        sp_view = sp[:, :].rearrange("p (r n) -> p r n", r=2)[:, :, 0:sw]
                        nc.scalar.activation(
                            e_view, sp_view, AF.Exp, scale=1.0 / math.sqrt(HD)
                        )
                        p1 = pen_sb[:, win["pen_off"] : win["pen_off"] + sw]
                        pen_b = AP(
                            tensor=p1.tensor, offset=p1.offset,
                            ap=[p1.ap[0], [0, 2], p1.ap[1]],
                        )
                        nc.vector.tensor_mul(e_view, e_view, pen_b)
                        if len(wq_q) > LAG:
                            emit_pv(*wq_q.pop(0))
                        for _ in range(2):
                            if fillers:
                                fsp = spp.tile([128, 1024], F32, tag="sp")
                                fillers.pop()(fsp)
                        wq_q.append((win, e))
                    for item in wq_q:
                        emit_pv(*item)
                    if pair == 0 and g == 1:
                        while fillers:   # drain before pair 1 needs dt1
                            fsp = spp.tile([128, 1024], F32, tag="sp")
                            fillers.pop()(fsp)

                    for h in range(2):
                        hg = 2 * pair + h
                        ctxT = cpool.tile([VW, GW], BF16, tag="ctxT")
                        nc.vector.tensor_scalar(
                            ctxT[:, :], pv[h][:, :], bvp_sb[:, hg : hg + 1],
                            None, OP.add,
                        )
                        nc.sync.dma_start(
                            out[hg * VW : (hg + 1) * VW, g * GW : (g + 1) * GW],
                            ctxT[:, :],
                        )


# ---------------------------------------------------------------- host

_NC_CACHE = {}


def _get_nc(plan):
    key = hash(plan["sig"])
    if key not in _NC_CACHE:
        _NC_CACHE[key] = build_nc(plan)
    return _NC_CACHE[key]


def _build_pen(plan, z):
    """Packed penalties [B][128, PENW] bf16 in window layout."""
    t = plan["t"]
    pens = []
    for b in range(B):
        perm = plan["perms"][b]
        zb = z[b]
        tb = t[b]
        buf = np.zeros((128, plan["penw"]), np.float64)
        for g in range(2):
            for win in plan["sched"][g]:
                off = win["pen_off"]
                for sg in win["segs"]:
                    kt = sg["kt"]
                    j = (kt * 128 + np.arange(128))[:, None]      # keys
                    zj = zb[kt * 128 : kt * 128 + 128][:, None]
                    qs = perm[sg["A"] : sg["B"]]
                    zq = zb[qs][None, :]
                    res = (1.0 - zq) * np.maximum(1.0 - zq - zj, 0.0) + \
                        zq * np.minimum(1.0 - zq + zj, 1.0)
                    scope = np.clip(tb[qs][None, :] - np.abs(qs[None, :] - j), 0.0, 1.0)
                    w = sg["B"] - sg["A"]
                    buf[:, off + sg["off"] : off + sg["off"] + w] = res * scope
        pens.append(buf.astype(ml_dtypes.bfloat16))
    return pens


def _prep_inputs(plan, hidden_states, Wq, bq, Wk, bk, Wv, bv, Wg, bg):
    f16 = np.float16
    hidden = np.asarray(hidden_states, np.float32)
    z = _gate_z(hidden, Wg, bg)
    pens = _build_pen(plan, z)

    Wq_f = np.asarray(Wq, np.float32)
    Wk_f = np.asarray(Wk, np.float32)
    Wv_f = np.asarray(Wv, np.float32)
    bq_f = np.asarray(bq, np.float32)
    bk_f = np.asarray(bk, np.float32)
    bv_f = np.asarray(bv, np.float32)

    in_maps = []
    for c in range(NC):
        b = c // 4
        hg = c % 4
        d0 = 256 * hg
        hT_f = hidden[b].T.astype(f16)                     # [H, S]
        hTq_f = np.ascontiguousarray(hT_f[:, plan["perms"][b]])

        def pack_w(Wf, width):
            # [(dt, ht), 128 rows, width cols] contiguous
            blocks = []
            ndt = 256 // width
            for dt in range(ndt):
                for ht in range(8):
                    blocks.append(
                        Wf[128 * ht : 128 * ht + 128, d0 + width * dt : d0 + width * (dt + 1)]
                    )
            return np.ascontiguousarray(np.concatenate(blocks, 0)).astype(f16)

        bkq_v = np.concatenate(
            [bk_f[d0 : d0 + 256], bq_f[d0 : d0 + 256]]
        ).astype(np.float32)
        bvp_a = np.zeros((VW, 4), np.float32)
        bvp_a[0:HD, :] = bv_f[d0 : d0 + 256].reshape(4, HD).T

        in_maps.append(
            {
                "hT": hT_f,
                "hTq": hTq_f,
                "Wk": pack_w(Wk_f, 128),
                "Wq": pack_w(Wq_f, 128),
                "Wv": pack_w(Wv_f, 256),
                "bkq": bkq_v,
                "bvp": bvp_a,
                "pen": pens[b],
            }
        )
    return in_maps


def _unshard(plan, results):
    out = np.empty((B, S, H), np.float32)
    for c in range(NC):
        b = c // 4
        hg = c % 4
        o = np.asarray(results[c]["out"]).astype(np.float32).reshape(4, VW, S)
        ctx = o[:, 0:HD, :] / o[:, HD : HD + 1, :]          # [4, 64, S]
        ctx = ctx.transpose(2, 0, 1).reshape(S, 256)        # [S perm, 256]
        out[b][plan["perms"][b], 256 * hg : 256 * hg + 256] = ctx
    return out


def _run(inputs, trace=False):
    z = _gate_z(
        np.asarray(inputs["hidden_states"], np.float32), inputs["Wg"], inputs["bg"]
    )
    plan = _make_plan(z)
    nc = _get_nc(plan)
    in_maps = _prep_inputs(plan, **inputs)
    res = run_bass_kernel_spmd(nc, in_maps, core_ids=list(range(NC)), trace=trace)
    return _unshard(plan, res.results), res


def kernel(**inputs) -> np.ndarray:
    out, _ = _run(inputs)
    return out


# revision 34
# speedup vs baseline: 1.0049x; 1.0049x over previous
"""Trainium2 Bass kernel: BERT self-attention with granularity-gated sparse
penalties, exploiting the data-dependent banded mask.

Math: softmax(S/8 + log(max(pen,1e-32))) == pen*exp(S/8) / sum(pen*exp(S/8)).
pen = res*scope with scope = clip(w_q+2 - |i-j|, 0, 1): a per-query BAND of
half-width w_q+2 = (S-2)^(1-z_q)+2.  ~88% of (q,k) pairs are exactly masked.

Sharding (8 cores): core c -> batch c//4, heads 4*(c%4)..+4 (dims 256*(c%4)).
K/V/Q projections computed only for the core's 256 dims (no redundancy).

Sparsity schedule (host, data-dependent, compiled per input):
 - queries sorted by band-width rank into chunks of CW=256 (index-sorted
   inside); per (chunk, key-tile kt) the active queries form a contiguous
   segment [A,B) after monotone closure (A,B non-decreasing in kt).
 - segments are the UNION over both batches -> identical program structure
   for all 8 cores (SPMD); extra columns self-zero via that batch's pen.
 - scores/exp/pen-mul/PV run only on segment columns (~25% of dense).

Per (pair of heads, group of 1024 queries): kt-loop; scores [128k x W] into
sp PSUM ([h0 bank | h1 bank]); one exp (ACT) per window; one pen-mul (DVE,
broadcast over the 2 heads); PV accumulates V^T@E into pv [65, 1024] PSUM
using per-byte pending-zero semantics (split at B_prev, bank start/stop at
first/last touch).  l = ones column 65 of V; host divides and un-permutes.

Penalties are host-precomputed (0.03% of FLOPs) packed [128, PENW] bf16 in
window order and DMA'd; gate z computed host-side in f64.
"""

import math

import ml_dtypes
import numpy as np

import concourse.bass as bass
import concourse.tile as tile
from concourse import bacc, mybir
from concourse.bass import AP
from concourse.bass_utils import run_bass_kernel_spmd

F32 = mybir.dt.float32
BF16 = mybir.dt.bfloat16
FP16 = mybir.dt.float16
AF = mybir.ActivationFunctionType
OP = mybir.AluOpType

B, S, H = 2, 2048, 1024
NH, HD = 16, 64
NC = 8
KT = 16            # key tiles of 128
CW = 512           # closure chunk width (queries) == one pv PSUM bank
NCH = S // CW      # 4 chunks
VW = HD + 1        # 65: V dims + ones column
GW = 1024          # query group width (pv tile)
WCAP = 512         # max window cols per head (one PSUM bank)
LAG = 3
LN_BASE = float(np.log(np.float32(S - 2)))


# ---------------------------------------------------------------- planning

def _gate_z(hidden, Wg, bg):
    Wg_f = np.asarray(Wg, np.float64).reshape(H)
    bg_f = float(np.asarray(bg, np.float64).reshape(()))
    pre = np.asarray(hidden, np.float64).reshape(B * S, H) @ Wg_f + bg_f
    return (1.0 / (1.0 + np.exp(-pre))).reshape(B, S)


def _make_plan(z):
    """Common (union over batches) sparse schedule + per-batch permutations."""
    idx = np.arange(S)
    t = np.exp((1.0 - z) * LN_BASE) + 2.0   # band half-width per query
    perms = []
    segs_b = np.full((B, NCH, KT, 2), -1, np.int64)
    for b in range(B):
        rank = np.argsort(np.argsort(t[b], kind="stable"), kind="stable")
        perm = np.lexsort((idx, rank // CW))
        perms.append(perm)
        lo = np.clip(np.floor(idx - t[b]) - 1, 0, S - 1).astype(np.int64)
        hi = np.clip(np.ceil(idx + t[b]) + 1, 0, S - 1).astype(np.int64)
        ktlo = lo[perm] // 128
        kthi = hi[perm] // 128
        for c in range(NCH):
            sl = slice(c * CW, (c + 1) * CW)
            kl = np.minimum.accumulate(ktlo[sl][::-1])[::-1]
            kh = np.maximum.accumulate(kthi[sl])
            for kt in range(KT):
                act = np.nonzero((kl <= kt) & (kh >= kt))[0]
                if len(act):
                    segs_b[b, c, kt] = (c * CW + act.min(), c * CW + act.max() + 1)

    # union across batches, then monotone closure (A suffix-min, B prefix-max)
    chunk_segs = []   # per chunk: list of (kt, A, B, Bprev)
    for c in range(NCH):
        A = np.full(KT, 1 << 30, np.int64)
        Bn = np.full(KT, -1, np.int64)
        for b in range(B):
            for kt in range(KT):
                a, bb = segs_b[b, c, kt]
                if a >= 0:
                    A[kt] = min(A[kt], a)
                    Bn[kt] = max(Bn[kt], bb)
        ne = np.nonzero(Bn >= 0)[0]
        k0, k1 = int(ne.min()), int(ne.max())
        for kt in range(k1 - 1, k0 - 1, -1):
            A[kt] = min(A[kt], A[kt + 1])
        for kt in range(k0 + 1, k1 + 1):
            Bn[kt] = max(Bn[kt], Bn[kt - 1])
        segs = []
        for kt in range(k0, k1 + 1):
            if A[kt] < Bn[kt]:
                bprev = int(Bn[kt - 1]) if kt > k0 else int(A[kt])
                bprev = min(max(bprev, int(A[kt])), int(Bn[kt]))
                segs.append((kt, int(A[kt]), int(Bn[kt]), bprev))
        chunk_segs.append(segs)

    # windows per group: kt-ordered segments packed across kt up to WCAP
    # cols per head (one sp PSUM bank); each seg carries its own kt.
    sched = [[] for _ in range(2)]      # sched[g] = [win], win = dict
    for g in range(2):
        segs = []
        for kt in range(KT):
            for c in range(2 * g, 2 * g + 2):
                for (skt, a, bb, bp) in chunk_segs[c]:
                    if skt == kt:
                        segs.append({"kt": kt, "c": c, "A": a, "B": bb})
        cur = []
        cw = 0
        for sg in segs:
            w = sg["B"] - sg["A"]
            if cur and cw + w > WCAP:
                sched[g].append({"segs": cur, "sw": cw})
                cur, cw = [], 0
            sg["off"] = cw
            cur.append(sg)
            cw += w
        if cur:
            sched[g].append({"segs": cur, "sw": cw})

    # pen offsets, interleaving groups by window start-kt so the pen DMA
    # stream is consumed roughly front-to-back
    off = 0
    for win in sorted(
        [w for g in range(2) for w in sched[g]], key=lambda w: w["segs"][0]["kt"]
    ):
        win["pen_off"] = off
        off += win["sw"]
    penw = off + (-off) % 8

    # PSUM bank start/stop flags: one PV matmul per (seg, head); HW per-byte
    # pending-zero handles mixed fresh/accumulate ranges.  start=True (bank
    # zeroing) only on the chunk-bank's first touch, stop on its last.
    for g in range(2):
        for c in range(2 * g, 2 * g + 2):
            seq = []
            for win in sched[g]:
                for sg in win["segs"]:
                    if sg["c"] == c:
                        sg["st"] = False
                        sg["sp"] = False
                        seq.append(sg)
            seq[0]["st"] = True
            seq[-1]["sp"] = True

    sig = (penw, tuple(
        (g, tuple((sg["kt"], sg["c"], sg["A"], sg["B"], sg["st"], sg["sp"])
                  for sg in win["segs"]))
        for g in range(2) for win in sched[g]
    ))
    return {"perms": perms, "sched": sched, "penw": penw, "t": t, "sig": sig}


# ---------------------------------------------------------------- device

def _pm_view(ap_1d, n_free):
    return AP(tensor=ap_1d.tensor, offset=ap_1d.offset, ap=[[1, 128], [128, n_free]])


def _wview(w2d, dt, width):
    return AP(
        tensor=w2d.tensor,
        offset=w2d.offset + dt * 8 * 128 * width,
        ap=[[width, 128], [128 * width, 8], [1, width]],
    )


def build_nc(plan):
    nc = bacc.Bacc("TRN2", target_bir_lowering=False, debug=False)
    penw = plan["penw"]
    hT = nc.dram_tensor("hT", [H, S], FP16, kind="ExternalInput").ap()
    hTq = nc.dram_tensor("hTq", [H, S], FP16, kind="ExternalInput").ap()
    Wk = nc.dram_tensor("Wk", [2 * 8 * 128, 128], FP16, kind="ExternalInput").ap()
    Wq = nc.dram_tensor("Wq", [2 * 8 * 128, 128], FP16, kind="ExternalInput").ap()
    Wv = nc.dram_tensor("Wv", [8 * 128, 256], FP16, kind="ExternalInput").ap()
    bkq = nc.dram_tensor("bkq", [4 * 128], F32, kind="ExternalInput").ap()
    bvp = nc.dram_tensor("bvp", [VW, 4], F32, kind="ExternalInput").ap()
    pen = nc.dram_tensor("pen", [128, penw], BF16, kind="ExternalInput").ap()
    out = nc.dram_tensor("out", [4 * VW, S], BF16, kind="ExternalOutput").ap()

    with tile.TileContext(nc) as tc:
        _body(tc, nc, plan, hT, hTq, Wk, Wq, Wv, bkq, bvp, pen, out)
    nc.compile()
    return nc


def _body(tc, nc, plan, hT, hTq, Wk, Wq, Wv, bkq, bvp, pen, out):
    import contextlib

    penw = plan["penw"]
    sched = plan["sched"]
    ctx = contextlib.ExitStack()
    with ctx:
        big = ctx.enter_context(tc.tile_pool(name="big", bufs=1))
        wpool = ctx.enter_context(tc.tile_pool(name="wp", bufs=1))
        epool = ctx.enter_context(tc.tile_pool(name="ep", bufs=7))
        cpool = ctx.enter_context(tc.tile_pool(name="cp", bufs=4))

        hT_ts = [big.tile([128, S], FP16, name=f"hT{i}", tag=f"hT{i}") for i in range(8)]
        hTq_ts = [
            big.tile([128, S], FP16, name=f"hTq{i}", tag=f"hTq{i}") for i in range(8)
        ]
        ktT = big.tile([128, 2 * S], FP16)
        qT = big.tile([128, 2 * S], FP16)
        v_sb = big.tile([128, KT * 4 * VW], BF16)
        pen_sb = big.tile([128, penw], BF16)
        bkq_sb = big.tile([128, 4], F32)
        bvp_sb = big.tile([VW, 4], F32)

        # --- DMA plan (measured queue rates: sync ~155GB/s, gpsimd ~109,
        # scalar ~23): weights first on sync, hidden interleaved over the
        # two fast queues (hT before hTq so K/V chase immediately), pen
        # split into 4 kt-progressive chunks.
        wkb = wpool.tile([128, 2, 8, 128], FP16, name="wkb")
        wqb = wpool.tile([128, 2, 8, 128], FP16, name="wqb")
        wvb = wpool.tile([128, 8, 256], FP16, name="wvb")
        ones_view = v_sb[:, :].rearrange("p (k c) -> p k c", c=VW)[:, :, HD : HD + 1]
        nc.gpsimd.memset(ones_view, 1.0)

        pq = penw // 2
        for dt in range(2):
            nc.sync.dma_start(wkb[:, dt, :, :], _wview(Wk, dt, 128))
        nc.sync.dma_start(wvb[:, :, :], _wview(Wv, 0, 256))
        for ht in range(8):
            nc.sync.dma_start(hT_ts[ht][:, :], hT[ht * 128 : ht * 128 + 128, :])
        nc.sync.dma_start(pen_sb[:, 0:pq], pen[:, 0:pq])
        for dt in range(2):
            nc.scalar.dma_start(wqb[:, dt, :, :], _wview(Wq, dt, 128))
        for ht in range(8):
            nc.scalar.dma_start(hTq_ts[ht][:, :], hTq[ht * 128 : ht * 128 + 128, :])
        nc.scalar.dma_start(pen_sb[:, pq:], pen[:, pq:])
        nc.gpsimd.dma_start(bkq_sb[:, :], _pm_view(bkq, 4))
        nc.gpsimd.dma_start(bvp_sb[:, :], bvp[:, :])

        # shared projection-unit bodies (psum arg: [128, >=512] f32 tile)
        def k_unit(ps, dt, tt, dve=False):
            for ht in range(8):
                nc.tensor.matmul(
                    ps[:, 0:512],
                    wkb[:, dt, ht, :],
                    hT_ts[ht][:, tt * 512 : (tt + 1) * 512],
                    start=(ht == 0),
                    stop=(ht == 7),
                )
            dst = ktT[:, dt * S + tt * 512 : dt * S + (tt + 1) * 512]
            if dve:
                nc.vector.tensor_scalar(dst, ps[:, 0:512], bkq_sb[:, dt : dt + 1], None, OP.add)
            else:
                nc.scalar.activation(dst, ps[:, 0:512], AF.Identity, bias=bkq_sb[:, dt : dt + 1])

        def q_unit(ps, dt, tt, dve=False):
            for ht in range(8):
                nc.tensor.matmul(
                    ps[:, 0:512],
                    wqb[:, dt, ht, :],
                    hTq_ts[ht][:, tt * 512 : (tt + 1) * 512],
                    start=(ht == 0),
                    stop=(ht == 7),
                )
            dst = qT[:, dt * S + tt * 512 : dt * S + (tt + 1) * 512]
            if dve:
                nc.vector.tensor_scalar(dst, ps[:, 0:512], bkq_sb[:, 2 + dt : 3 + dt], None, OP.add)
            else:
                nc.scalar.activation(dst, ps[:, 0:512], AF.Identity, bias=bkq_sb[:, 2 + dt : 3 + dt])

        def v_unit(ps, tv):
            for ht in range(8):
                nc.tensor.matmul(
                    ps[:, 0:256],
                    hT_ts[ht][:, tv * 128 : tv * 128 + 128],
                    wvb[:, ht, :],
                    start=(ht == 0),
                    stop=(ht == 7),
                )
            base = tv * 4 * VW
            dst = v_sb[:, base : base + 4 * VW].rearrange(
                "p (h c) -> p h c", c=VW
            )[:, :, 0:HD]
            src = ps[:, 0:256].rearrange("p (h c) -> p h c", c=HD)
            nc.vector.tensor_scalar(dst, src, 0.0, None, OP.add)

        # --- prefix: K-dt0 (chases hT DMA), V kt 0-3, Q-dt0 (needs hTq)
        with tc.tile_pool(name="pp", bufs=1, space="PSUM") as pp:
            kps = [pp.tile([128, 512], F32, tag=f"a{i}", name=f"a{i}") for i in range(4)]
            qps = [pp.tile([128, 512], F32, tag=f"b{i}", name=f"b{i}") for i in range(4)]
            for ht in range(8):
                for tt in range(4):
                    nc.tensor.matmul(
                        kps[tt][:, :],
                        wkb[:, 0, ht, :],
                        hT_ts[ht][:, tt * 512 : (tt + 1) * 512],
                        start=(ht == 0),
                        stop=(ht == 7),
                    )
            for tt in range(4):
                nc.scalar.activation(
                    ktT[:, tt * 512 : (tt + 1) * 512],
                    kps[tt][:, :],
                    AF.Identity,
                    bias=bkq_sb[:, 0:1],
                )
            for tv in range(4):
                ps = pp.tile([128, 512], F32, tag=f"a{tv}", name=f"av{tv}")
                v_unit(ps, tv)
            for ht in range(8):
                for tt in range(4):
                    nc.tensor.matmul(
                        qps[tt][:, :],
                        wqb[:, 0, ht, :],
                        hTq_ts[ht][:, tt * 512 : (tt + 1) * 512],
                        start=(ht == 0),
                        stop=(ht == 7),
                    )
            for tt in range(4):
                nc.scalar.activation(
                    qT[:, tt * 512 : (tt + 1) * 512],
                    qps[tt][:, :],
                    AF.Identity,
                    bias=bkq_sb[:, 2:3],
                )

        # --- attention: pairs sequential; per (pair, group) a kt loop.
        # Remaining projections (V 4-15, K-dt1, Q-dt1) run as FILLER units
        # inside pair-0's window rotation (same sp PSUM pool) so the PE
        # stream never drains and stays at max p-state.
        fillers = [lambda ps, tv=tv: v_unit(ps, tv) for tv in range(4, KT)]
        fillers += [lambda ps, tt=tt: k_unit(ps, 1, tt, dve=True) for tt in range(4)]
        fillers += [lambda ps, tt=tt: q_unit(ps, 1, tt, dve=True) for tt in range(4)]
        fillers = fillers[::-1]          # pop() from the front

        with (
            tc.tile_pool(name="sp", bufs=2, space="PSUM") as spp,
            tc.tile_pool(name="pvp", bufs=1, space="PSUM") as pvp,
        ):
            for pair in range(2):
                for g in range(2):
                    pv = [
                        pvp.tile([VW, GW], F32, tag=f"pv{h}", name=f"pv{h}")
                        for h in range(2)
                    ]

                    def emit_pv(win, e, pair=pair, g=g, pv=pv):
                        sw = win["sw"]
                        for sg in win["segs"]:
                            kt = sg["kt"]
                            for h in range(2):
                                lhs = v_sb[
                                    :,
                                    kt * 4 * VW + (2 * pair + h) * VW :
                                    kt * 4 * VW + (2 * pair + h) * VW + VW,
                                ]
                                eoff = h * sw + sg["off"]
                                nc.tensor.matmul(
                                    pv[h][:, sg["A"] - GW * g : sg["B"] - GW * g],
                                    lhs,
                                    e[:, eoff : eoff + (sg["B"] - sg["A"])],
                                    start=sg["st"],
                                    stop=sg["sp"],
                                    skip_group_check=True,
                                )

                    wq_q = []
                    for win in sched[g]:
                        sw = win["sw"]
                        sp = spp.tile([128, 1024], F32, tag="sp")
                        nseg = len(win["segs"])
                        for si, sg in enumerate(win["segs"]):
                            a, bb, kt = sg["A"], sg["B"], sg["kt"]
                            for h in range(2):
                                nc.tensor.matmul(
                                    sp[:, h * 512 + sg["off"] : h * 512 + sg["off"] + (bb - a)],
                                    ktT[
                                        h * 64 : h * 64 + 64,
                                        pair * S + kt * 128 : pair * S + kt * 128 + 128,
                                    ],
                                    qT[h * 64 : h * 64 + 64, pair * S + a : pair * S + bb],
                                    start=(si == 0),
                                    stop=(si == nseg - 1),
                                    tile_position=(h * 64, 0),
                                )
                        e = epool.tile([128, 1024], BF16, tag="e")
                        e_view = e[:, 0 : 2 * sw].rearrange("p (r n) -> p r n", r=2)
                        sp_view = sp[:, :].rearrange("p (r n) -> p r n", r=2)[:, :, 0:sw]
                        nc.scalar.activation(
                            e_view, sp_view, AF.Exp, scale=1.0 / math.sqrt(HD)
                        )
                        p1 = pen_sb[:, win["pen_off"] : win["pen_off"] + sw]
                        pen_b = AP(
                            tensor=p1.tensor, offset=p1.offset,
                            ap=[p1.ap[0], [0, 2], p1.ap[1]],
                        )
                        nc.vector.tensor_mul(e_view, e_view, pen_b)
                        if len(wq_q) > LAG:
                            emit_pv(*wq_q.pop(0))
                        for _ in range(2):
                            if fillers:
                                fsp = spp.tile([128, 1024], F32, tag="sp")
                                fillers.pop()(fsp)
                        wq_q.append((win, e))
                    for item in wq_q:
                        emit_pv(*item)
                    if pair == 0 and g == 1:
                        while fillers:   # drain before pair 1 needs dt1
                            fsp = spp.tile([128, 1024], F32, tag="sp")
                            fillers.pop()(fsp)

                    for h in range(2):
                        hg = 2 * pair + h
                        ctxT = cpool.tile([VW, GW], BF16, tag="ctxT")
                        nc.vector.tensor_scalar(
                            ctxT[:, :], pv[h][:, :], bvp_sb[:, hg : hg + 1],
                            None, OP.add,
                        )
                        nc.sync.dma_start(
                            out[hg * VW : (hg + 1) * VW, g * GW : (g + 1) * GW],
                            ctxT[:, :],
                        )


# ---------------------------------------------------------------- host

_NC_CACHE = {}


def _get_nc(plan):
    key = hash(plan["sig"])
    if key not in _NC_CACHE:
        _NC_CACHE[key] = build_nc(plan)
    return _NC_CACHE[key]


def _build_pen(plan, z):
    """Packed penalties [B][128, PENW] bf16 in window layout."""
    t = plan["t"]
    pens = []
    for b in range(B):
        perm = plan["perms"][b]
        zb = z[b]
        tb = t[b]
        buf = np.zeros((128, plan["penw"]), np.float64)
        for g in range(2):
            for win in plan["sched"][g]:
                off = win["pen_off"]
                for sg in win["segs"]:
                    kt = sg["kt"]
                    j = (kt * 128 + np.arange(128))[:, None]      # keys
                    zj = zb[kt * 128 : kt * 128 + 128][:, None]
                    qs = perm[sg["A"] : sg["B"]]
                    zq = zb[qs][None, :]
                    res = (1.0 - zq) * np.maximum(1.0 - zq - zj, 0.0) + \
                        zq * np.minimum(1.0 - zq + zj, 1.0)
                    scope = np.clip(tb[qs][None, :] - np.abs(qs[None, :] - j), 0.0, 1.0)
                    w = sg["B"] - sg["A"]
                    buf[:, off + sg["off"] : off + sg["off"] + w] = res * scope
        pens.append(buf.astype(ml_dtypes.bfloat16))
    return pens


def _prep_inputs(plan, hidden_states, Wq, bq, Wk, bk, Wv, bv, Wg, bg):
    f16 = np.float16
    hidden = np.asarray(hidden_states, np.float32)
    z = _gate_z(hidden, Wg, bg)
    pens = _build_pen(plan, z)

    Wq_f = np.asarray(Wq, np.float32)
    Wk_f = np.asarray(Wk, np.float32)
    Wv_f = np.asarray(Wv, np.float32)
    bq_f = np.asarray(bq, np.float32)
    bk_f = np.asarray(bk, np.float32)
    bv_f = np.asarray(bv, np.float32)

    in_maps = []
    for c in range(NC):
        b = c // 4
        hg = c % 4
        d0 = 256 * hg
        hT_f = hidden[b].T.astype(f16)                     # [H, S]
        hTq_f = np.ascontiguousarray(hT_f[:, plan["perms"][b]])

        def pack_w(Wf, width):
            # [(dt, ht), 128 rows, width cols] contiguous
            blocks = []
            ndt = 256 // width
            for dt in range(ndt):
                for ht in range(8):
                    blocks.append(
                        Wf[128 * ht : 128 * ht + 128, d0 + width * dt : d0 + width * (dt + 1)]
                    )
            return np.ascontiguousarray(np.concatenate(blocks, 0)).astype(f16)

        bkq_v = np.concatenate(
            [bk_f[d0 : d0 + 256], bq_f[d0 : d0 + 256]]
        ).astype(np.float32)
        bvp_a = np.zeros((VW, 4), np.float32)
        bvp_a[0:HD, :] = bv_f[d0 : d0 + 256].reshape(4, HD).T

        in_maps.append(
            {
                "hT": hT_f,
                "hTq": hTq_f,
                "Wk": pack_w(Wk_f, 128),
                "Wq": pack_w(Wq_f, 128),
                "Wv": pack_w(Wv_f, 256),
                "bkq": bkq_v,
                "bvp": bvp_a,
                "pen": pens[b],
            }
        )
    return in_maps


def _unshard(plan, results):
    out = np.empty((B, S, H), np.float32)
    for c in range(NC):
        b = c // 4
        hg = c % 4
        o = np.asarray(results[c]["out"]).astype(np.float32).reshape(4, VW, S)
        ctx = o[:, 0:HD, :] / o[:, HD : HD + 1, :]          # [4, 64, S]
        ctx = ctx.transpose(2, 0, 1).reshape(S, 256)        # [S perm, 256]
        out[b][plan["perms"][b], 256 * hg : 256 * hg + 256] = ctx
    return out


def _run(inputs, trace=False):
    z = _gate_z(
        np.asarray(inputs["hidden_states"], np.float32), inputs["Wg"], inputs["bg"]
    )
    plan = _make_plan(z)
    nc = _get_nc(plan)
    in_maps = _prep_inputs(plan, **inputs)
    res = run_bass_kernel_spmd(nc, in_maps, core_ids=list(range(NC)), trace=trace)
    return _unshard(plan, res.results), res


def kernel(**inputs) -> np.ndarray:
    out, _ = _run(inputs)
    return out


# revision 36
# speedup vs baseline: 1.0133x; 1.0083x over previous
"""Trainium2 Bass kernel: BERT self-attention with granularity-gated sparse
penalties, exploiting the data-dependent banded mask.

Math: softmax(S/8 + log(max(pen,1e-32))) == pen*exp(S/8) / sum(pen*exp(S/8)).
pen = res*scope with scope = clip(w_q+2 - |i-j|, 0, 1): a per-query BAND of
half-width w_q+2 = (S-2)^(1-z_q)+2.  ~88% of (q,k) pairs are exactly masked.

Sharding (8 cores): core c -> batch c//4, heads 4*(c%4)..+4 (dims 256*(c%4)).
K/V/Q projections computed only for the core's 256 dims (no redundancy).

Sparsity schedule (host, data-dependent, compiled per input):
 - queries sorted by band-width rank into chunks of CW=256 (index-sorted
   inside); per (chunk, key-tile kt) the active queries form a contiguous
   segment [A,B) after monotone closure (A,B non-decreasing in kt).
 - segments are the UNION over both batches -> identical program structure
   for all 8 cores (SPMD); extra columns self-zero via that batch's pen.
 - scores/exp/pen-mul/PV run only on segment columns (~25% of dense).

Per (pair of heads, group of 1024 queries): kt-loop; scores [128k x W] into
sp PSUM ([h0 bank | h1 bank]); one exp (ACT) per window; one pen-mul (DVE,
broadcast over the 2 heads); PV accumulates V^T@E into pv [65, 1024] PSUM
using per-byte pending-zero semantics (split at B_prev, bank start/stop at
first/last touch).  l = ones column 65 of V; host divides and un-permutes.

Penalties are host-precomputed (0.03% of FLOPs) packed [128, PENW] bf16 in
window order and DMA'd; gate z computed host-side in f64.
"""

import math

import ml_dtypes
import numpy as np

import concourse.bass as bass
import concourse.tile as tile
from concourse import bacc, mybir
from concourse.bass import AP
from concourse.bass_utils import run_bass_kernel_spmd

F32 = mybir.dt.float32
BF16 = mybir.dt.bfloat16
FP16 = mybir.dt.float16
AF = mybir.ActivationFunctionType
OP = mybir.AluOpType

B, S, H = 2, 2048, 1024
NH, HD = 16, 64
NC = 8
KT = 16            # key tiles of 128
CW = 512           # closure chunk width (queries) == one pv PSUM bank
NCH = S // CW      # 4 chunks
VW = HD + 1        # 65: V dims + ones column
GW = 1024          # query group width (pv tile)
WCAP = 512         # max window cols per head (one PSUM bank)
LAG = 3
LN_BASE = float(np.log(np.float32(S - 2)))


# ---------------------------------------------------------------- planning

def _gate_z(hidden, Wg, bg):
    Wg_f = np.asarray(Wg, np.float64).reshape(H)
    bg_f = float(np.asarray(bg, np.float64).reshape(()))
    pre = np.asarray(hidden, np.float64).reshape(B * S, H) @ Wg_f + bg_f
    return (1.0 / (1.0 + np.exp(-pre))).reshape(B, S)


def _make_plan(z):
    """Common (union over batches) sparse schedule + per-batch permutations."""
    idx = np.arange(S)
    t = np.exp((1.0 - z) * LN_BASE) + 2.0   # band half-width per query
    perms = []
    segs_b = np.full((B, NCH, KT, 2), -1, np.int64)
    for b in range(B):
        rank = np.argsort(np.argsort(t[b], kind="stable"), kind="stable")
        perm = np.lexsort((idx, rank // CW))
        perms.append(perm)
        lo = np.clip(np.floor(idx - t[b]) - 1, 0, S - 1).astype(np.int64)
        hi = np.clip(np.ceil(idx + t[b]) + 1, 0, S - 1).astype(np.int64)
        ktlo = lo[perm] // 128
        kthi = hi[perm] // 128
        for c in range(NCH):
            sl = slice(c * CW, (c + 1) * CW)
            kl = np.minimum.accumulate(ktlo[sl][::-1])[::-1]
            kh = np.maximum.accumulate(kthi[sl])
            for kt in range(KT):
                act = np.nonzero((kl <= kt) & (kh >= kt))[0]
                if len(act):
                    segs_b[b, c, kt] = (c * CW + act.min(), c * CW + act.max() + 1)

    # union across batches, then monotone closure (A suffix-min, B prefix-max)
    chunk_segs = []   # per chunk: list of (kt, A, B, Bprev)
    for c in range(NCH):
        A = np.full(KT, 1 << 30, np.int64)
        Bn = np.full(KT, -1, np.int64)
        for b in range(B):
            for kt in range(KT):
                a, bb = segs_b[b, c, kt]
                if a >= 0:
                    A[kt] = min(A[kt], a)
                    Bn[kt] = max(Bn[kt], bb)
        ne = np.nonzero(Bn >= 0)[0]
        k0, k1 = int(ne.min()), int(ne.max())
        for kt in range(k1 - 1, k0 - 1, -1):
            A[kt] = min(A[kt], A[kt + 1])
        for kt in range(k0 + 1, k1 + 1):
            Bn[kt] = max(Bn[kt], Bn[kt - 1])
        segs = []
        for kt in range(k0, k1 + 1):
            if A[kt] < Bn[kt]:
                bprev = int(Bn[kt - 1]) if kt > k0 else int(A[kt])
                bprev = min(max(bprev, int(A[kt])), int(Bn[kt]))
                segs.append((kt, int(A[kt]), int(Bn[kt]), bprev))
        chunk_segs.append(segs)

    # windows per group: kt-ordered segments packed across kt up to WCAP
    # cols per head (one sp PSUM bank); each seg carries its own kt.
    sched = [[] for _ in range(2)]      # sched[g] = [win], win = dict
    for g in range(2):
        segs = []
        for kt in range(KT):
            for c in range(2 * g, 2 * g + 2):
                for (skt, a, bb, bp) in chunk_segs[c]:
                    if skt == kt:
                        segs.append({"kt": kt, "c": c, "A": a, "B": bb})
        cur = []
        cw = 0
        for sg in segs:
            w = sg["B"] - sg["A"]
            if cur and cw + w > WCAP:
                sched[g].append({"segs": cur, "sw": cw})
                cur, cw = [], 0
            sg["off"] = cw
            cur.append(sg)
            cw += w
        if cur:
            sched[g].append({"segs": cur, "sw": cw})

    # pen offsets, interleaving groups by window start-kt so the pen DMA
    # stream is consumed roughly front-to-back
    off = 0
    for win in sorted(
        [w for g in range(2) for w in sched[g]], key=lambda w: w["segs"][0]["kt"]
    ):
        win["pen_off"] = off
        off += win["sw"]
    penw = off + (-off) % 8

    # PSUM bank start/stop flags: one PV matmul per (seg, head); HW per-byte
    # pending-zero handles mixed fresh/accumulate ranges.  start=True (bank
    # zeroing) only on the chunk-bank's first touch, stop on its last.
    for g in range(2):
        for c in range(2 * g, 2 * g + 2):
            seq = []
            for win in sched[g]:
                for sg in win["segs"]:
                    if sg["c"] == c:
                        sg["st"] = False
                        sg["sp"] = False
                        seq.append(sg)
            seq[0]["st"] = True
            seq[-1]["sp"] = True

    sig = (penw, tuple(
        (g, tuple((sg["kt"], sg["c"], sg["A"], sg["B"], sg["st"], sg["sp"])
                  for sg in win["segs"]))
        for g in range(2) for win in sched[g]
    ))
    return {"perms": perms, "sched": sched, "penw": penw, "t": t, "sig": sig}


# ---------------------------------------------------------------- device

def _pm_view(ap_1d, n_free):
    return AP(tensor=ap_1d.tensor, offset=ap_1d.offset, ap=[[1, 128], [128, n_free]])


def _wview(w2d, dt, width):
    return AP(
        tensor=w2d.tensor,
        offset=w2d.offset + dt * 8 * 128 * width,
        ap=[[width, 128], [128 * width, 8], [1, width]],
    )


def build_nc(plan):
    nc = bacc.Bacc("TRN2", target_bir_lowering=False, debug=False)
    penw = plan["penw"]
    hT = nc.dram_tensor("hT", [H, S], FP16, kind="ExternalInput").ap()
    hTq = nc.dram_tensor("hTq", [H, S], FP16, kind="ExternalInput").ap()
    Wk = nc.dram_tensor("Wk", [2 * 8 * 128, 128], FP16, kind="ExternalInput").ap()
    Wq = nc.dram_tensor("Wq", [2 * 8 * 128, 128], FP16, kind="ExternalInput").ap()
    Wv = nc.dram_tensor("Wv", [8 * 128, 256], FP16, kind="ExternalInput").ap()
    bkq = nc.dram_tensor("bkq", [4 * 128], F32, kind="ExternalInput").ap()
    bvp = nc.dram_tensor("bvp", [VW, 4], F32, kind="ExternalInput").ap()
    pen = nc.dram_tensor("pen", [128, penw], BF16, kind="ExternalInput").ap()
    out = nc.dram_tensor("out", [4 * VW, S], BF16, kind="ExternalOutput").ap()

    with tile.TileContext(nc) as tc:
        _body(tc, nc, plan, hT, hTq, Wk, Wq, Wv, bkq, bvp, pen, out)
    nc.compile()
    return nc


def _body(tc, nc, plan, hT, hTq, Wk, Wq, Wv, bkq, bvp, pen, out):
    import contextlib

    penw = plan["penw"]
    sched = plan["sched"]
    ctx = contextlib.ExitStack()
    with ctx:
        big = ctx.enter_context(tc.tile_pool(name="big", bufs=1))
        wpool = ctx.enter_context(tc.tile_pool(name="wp", bufs=1))
        epool = ctx.enter_context(tc.tile_pool(name="ep", bufs=7))
        cpool = ctx.enter_context(tc.tile_pool(name="cp", bufs=4))

        hT_ts = [big.tile([128, S], FP16, name=f"hT{i}", tag=f"hT{i}") for i in range(8)]
        hTq_ts = [
            big.tile([128, S], FP16, name=f"hTq{i}", tag=f"hTq{i}") for i in range(8)
        ]
        ktT = big.tile([128, 2 * S], FP16)
        qT = big.tile([128, 2 * S], FP16)
        v_sb = big.tile([128, KT * 4 * VW], BF16)
        pen_sb = big.tile([128, penw], BF16)
        bkq_sb = big.tile([128, 4], F32)
        bvp_sb = big.tile([VW, 4], F32)

        # --- DMA plan (measured queue rates: sync ~155GB/s, gpsimd ~109,
        # scalar ~23): weights first on sync, hidden interleaved over the
        # two fast queues (hT before hTq so K/V chase immediately), pen
        # split into 4 kt-progressive chunks.
        wkb = wpool.tile([128, 2, 8, 128], FP16, name="wkb")
        wqb = wpool.tile([128, 2, 8, 128], FP16, name="wqb")
        wvb = wpool.tile([128, 8, 256], FP16, name="wvb")
        ones_view = v_sb[:, :].rearrange("p (k c) -> p k c", c=VW)[:, :, HD : HD + 1]
        nc.gpsimd.memset(ones_view, 1.0)

        pq = penw // 2
        for dt in range(2):
            nc.sync.dma_start(wkb[:, dt, :, :], _wview(Wk, dt, 128))
        nc.sync.dma_start(wvb[:, :, :], _wview(Wv, 0, 256))
        for ht in range(8):
            nc.sync.dma_start(hT_ts[ht][:, :], hT[ht * 128 : ht * 128 + 128, :])
        nc.sync.dma_start(pen_sb[:, 0:pq], pen[:, 0:pq])
        for dt in range(2):
            nc.scalar.dma_start(wqb[:, dt, :, :], _wview(Wq, dt, 128))
        for ht in range(8):
            nc.scalar.dma_start(hTq_ts[ht][:, :], hTq[ht * 128 : ht * 128 + 128, :])
        nc.scalar.dma_start(pen_sb[:, pq:], pen[:, pq:])
        nc.gpsimd.dma_start(bkq_sb[:, :], _pm_view(bkq, 4))
        nc.gpsimd.dma_start(bvp_sb[:, :], bvp[:, :])

        # shared projection-unit bodies (psum arg: [128, >=512] f32 tile)
        def k_unit(ps, dt, tt, dve=False):
            for ht in range(8):
                nc.tensor.matmul(
                    ps[:, 0:512],
                    wkb[:, dt, ht, :],
                    hT_ts[ht][:, tt * 512 : (tt + 1) * 512],
                    start=(ht == 0),
                    stop=(ht == 7),
                )
            dst = ktT[:, dt * S + tt * 512 : dt * S + (tt + 1) * 512]
            if dve:
                nc.vector.tensor_scalar(dst, ps[:, 0:512], bkq_sb[:, dt : dt + 1], None, OP.add)
            else:
                nc.scalar.activation(dst, ps[:, 0:512], AF.Identity, bias=bkq_sb[:, dt : dt + 1])

        def q_unit(ps, dt, tt, dve=False):
            for ht in range(8):
                nc.tensor.matmul(
                    ps[:, 0:512],
                    wqb[:, dt, ht, :],
                    hTq_ts[ht][:, tt * 512 : (tt + 1) * 512],
                    start=(ht == 0),
                    stop=(ht == 7),
                )
            dst = qT[:, dt * S + tt * 512 : dt * S + (tt + 1) * 512]
            if dve:
                nc.vector.tensor_scalar(dst, ps[:, 0:512], bkq_sb[:, 2 + dt : 3 + dt], None, OP.add)
            else:
                nc.scalar.activation(dst, ps[:, 0:512], AF.Identity, bias=bkq_sb[:, 2 + dt : 3 + dt])

        def v_unit(ps, tv):
            for ht in range(8):
                nc.tensor.matmul(
                    ps[:, 0:256],
                    hT_ts[ht][:, tv * 128 : tv * 128 + 128],
                    wvb[:, ht, :],
                    start=(ht == 0),
                    stop=(ht == 7),
                )
            base = tv * 4 * VW
            dst = v_sb[:, base : base + 4 * VW].rearrange(
                "p (h c) -> p h c", c=VW
            )[:, :, 0:HD]
            src = ps[:, 0:256].rearrange("p (h c) -> p h c", c=HD)
            nc.vector.tensor_scalar(dst, src, 0.0, None, OP.add)

        # --- prefix: K dt0+dt1 interleaved (fills the hT DMA chase), V kt 0-3,
        # Q-dt0 (needs full hTq)
        with tc.tile_pool(name="pp", bufs=1, space="PSUM") as pp:
            kps = [pp.tile([128, 512], F32, tag=f"a{i}", name=f"a{i}") for i in range(4)]
            qps = [pp.tile([128, 512], F32, tag=f"b{i}", name=f"b{i}") for i in range(4)]
            for ht in range(8):
                for tt in range(4):
                    nc.tensor.matmul(
                        kps[tt][:, :],
                        wkb[:, 0, ht, :],
                        hT_ts[ht][:, tt * 512 : (tt + 1) * 512],
                        start=(ht == 0),
                        stop=(ht == 7),
                    )
                for tt in range(4):
                    nc.tensor.matmul(
                        qps[tt][:, :],
                        wkb[:, 1, ht, :],
                        hT_ts[ht][:, tt * 512 : (tt + 1) * 512],
                        start=(ht == 0),
                        stop=(ht == 7),
                    )
            for tt in range(4):
                nc.scalar.activation(
                    ktT[:, tt * 512 : (tt + 1) * 512],
                    kps[tt][:, :],
                    AF.Identity,
                    bias=bkq_sb[:, 0:1],
                )
            for tt in range(4):
                nc.scalar.activation(
                    ktT[:, S + tt * 512 : S + (tt + 1) * 512],
                    qps[tt][:, :],
                    AF.Identity,
                    bias=bkq_sb[:, 1:2],
                )
            for tv in range(4):
                ps = pp.tile([128, 512], F32, tag=f"a{tv}", name=f"av{tv}")
                v_unit(ps, tv)
            for tt in range(4):
                qp = pp.tile([128, 512], F32, tag=f"b{tt}", name=f"bq{tt}")
                for ht in range(8):
                    nc.tensor.matmul(
                        qp[:, :],
                        wqb[:, 0, ht, :],
                        hTq_ts[ht][:, tt * 512 : (tt + 1) * 512],
                        start=(ht == 0),
                        stop=(ht == 7),
                    )
                nc.scalar.activation(
                    qT[:, tt * 512 : (tt + 1) * 512],
                    qp[:, :],
                    AF.Identity,
                    bias=bkq_sb[:, 2:3],
                )

        # --- attention: pairs sequential; per (pair, group) a kt loop.
        # Remaining projections (V 4-15, K-dt1, Q-dt1) run as FILLER units
        # inside pair-0's window rotation (same sp PSUM pool) so the PE
        # stream never drains and stays at max p-state.
        fillers = [lambda ps, tv=tv: v_unit(ps, tv) for tv in range(4, KT)]
        fillers += [lambda ps, tt=tt: q_unit(ps, 1, tt, dve=True) for tt in range(4)]
        fillers = fillers[::-1]          # pop() from the front

        with (
            tc.tile_pool(name="sp", bufs=2, space="PSUM") as spp,
            tc.tile_pool(name="pvp", bufs=1, space="PSUM") as pvp,
        ):
            for pair in range(2):
                for g in range(2):
                    pv = [
                        pvp.tile([VW, GW], F32, tag=f"pv{h}", name=f"pv{h}")
                        for h in range(2)
                    ]

                    def emit_pv(win, e, pair=pair, g=g, pv=pv):
                        sw = win["sw"]
                        for sg in win["segs"]:
                            kt = sg["kt"]
                            for h in range(2):
                                lhs = v_sb[
                                    :,
                                    kt * 4 * VW + (2 * pair + h) * VW :
                                    kt * 4 * VW + (2 * pair + h) * VW + VW,
                                ]
                                eoff = h * sw + sg["off"]
                                nc.tensor.matmul(
                                    pv[h][:, sg["A"] - GW * g : sg["B"] - GW * g],
                                    lhs,
                                    e[:, eoff : eoff + (sg["B"] - sg["A"])],
                                    start=sg["st"],
                                    stop=sg["sp"],
                                    skip_group_check=True,
                                )

                    wq_q = []
                    for win in sched[g]:
                        sw = win["sw"]
                        sp = spp.tile([128, 1024], F32, tag="sp")
                        nseg = len(win["segs"])
                        for si, sg in enumerate(win["segs"]):
                            a, bb, kt = sg["A"], sg["B"], sg["kt"]
                            for h in range(2):
                                nc.tensor.matmul(
                                    sp[:, h * 512 + sg["off"] : h * 512 + sg["off"] + (bb - a)],
                                    ktT[
                                        h * 64 : h * 64 + 64,
                                        pair * S + kt * 128 : pair * S + kt * 128 + 128,
                                    ],
                                    qT[h * 64 : h * 64 + 64, pair * S + a : pair * S + bb],
                                    start=(si == 0),
                                    stop=(si == nseg - 1),
                                    tile_position=(h * 64, 0),
                                )
                        e = epool.tile([128, 1024], BF16, tag="e")
                        e_view = e[:, 0 : 2 * sw].rearrange("p (r n) -> p r n", r=2)
                        sp_view = sp[:, :].rearrange("p (r n) -> p r n", r=2)[:, :, 0:sw]
                        nc.scalar.activation(
                            e_view, sp_view, AF.Exp, scale=1.0 / math.sqrt(HD)
                        )
                        p1 = pen_sb[:, win["pen_off"] : win["pen_off"] + sw]
                        pen_b = AP(
                            tensor=p1.tensor, offset=p1.offset,
                            ap=[p1.ap[0], [0, 2], p1.ap[1]],
                        )
                        nc.vector.tensor_mul(e_view, e_view, pen_b)
                        if len(wq_q) > LAG:
                            emit_pv(*wq_q.pop(0))
                        for _ in range(2):
                            if fillers:
                                fsp = spp.tile([128, 1024], F32, tag="sp")
                                fillers.pop()(fsp)
                        wq_q.append((win, e))
                    for item in wq_q:
                        emit_pv(*item)
                    if pair == 0 and g == 1:
                        while fillers:   # drain before pair 1 needs dt1
                            fsp = spp.tile([128, 1024], F32, tag="sp")
                            fillers.pop()(fsp)

                    for h in range(2):
                        hg = 2 * pair + h
                        ctxT = cpool.tile([VW, GW], BF16, tag="ctxT")
                        nc.vector.tensor_scalar(
                            ctxT[:, :], pv[h][:, :], bvp_sb[:, hg : hg + 1],
                            None, OP.add,
                        )
                        nc.sync.dma_start(
                            out[hg * VW : (hg + 1) * VW, g * GW : (g + 1) * GW],
                            ctxT[:, :],
                        )


# ---------------------------------------------------------------- host

_NC_CACHE = {}


def _get_nc(plan):
    key = hash(plan["sig"])
    if key not in _NC_CACHE:
        _NC_CACHE[key] = build_nc(plan)
    return _NC_CACHE[key]


def _build_pen(plan, z):
    """Packed penalties [B][128, PENW] bf16 in window layout."""
    t = plan["t"]
    pens = []
    for b in range(B):
        perm = plan["perms"][b]
        zb = z[b]
        tb = t[b]
        buf = np.zeros((128, plan["penw"]), np.float64)
        for g in range(2):
            for win in plan["sched"][g]:
                off = win["pen_off"]
                for sg in win["segs"]:
                    kt = sg["kt"]
                    j = (kt * 128 + np.arange(128))[:, None]      # keys
                    zj = zb[kt * 128 : kt * 128 + 128][:, None]
                    qs = perm[sg["A"] : sg["B"]]
                    zq = zb[qs][None, :]
                    res = (1.0 - zq) * np.maximum(1.0 - zq - zj, 0.0) + \
                        zq * np.minimum(1.0 - zq + zj, 1.0)
                    scope = np.clip(tb[qs][None, :] - np.abs(qs[None, :] - j), 0.0, 1.0)
                    w = sg["B"] - sg["A"]
                    buf[:, off + sg["off"] : off + sg["off"] + w] = res * scope
        pens.append(buf.astype(ml_dtypes.bfloat16))
    return pens


def _prep_inputs(plan, hidden_states, Wq, bq, Wk, bk, Wv, bv, Wg, bg):
    f16 = np.float16
    hidden = np.asarray(hidden_states, np.float32)
    z = _gate_z(hidden, Wg, bg)
    pens = _build_pen(plan, z)

    Wq_f = np.asarray(Wq, np.float32)
    Wk_f = np.asarray(Wk, np.float32)
    Wv_f = np.asarray(Wv, np.float32)
    bq_f = np.asarray(bq, np.float32)
    bk_f = np.asarray(bk, np.float32)
    bv_f = np.asarray(bv, np.float32)

    in_maps = []
    for c in range(NC):
        b = c // 4
        hg = c % 4
        d0 = 256 * hg
        hT_f = hidden[b].T.astype(f16)                     # [H, S]
        hTq_f = np.ascontiguousarray(hT_f[:, plan["perms"][b]])

        def pack_w(Wf, width):
            # [(dt, ht), 128 rows, width cols] contiguous
            blocks = []
            ndt = 256 // width
            for dt in range(ndt):
                for ht in range(8):
                    blocks.append(
                        Wf[128 * ht : 128 * ht + 128, d0 + width * dt : d0 + width * (dt + 1)]
                    )
            return np.ascontiguousarray(np.concatenate(blocks, 0)).astype(f16)

        bkq_v = np.concatenate(
            [bk_f[d0 : d0 + 256], bq_f[d0 : d0 + 256]]
        ).astype(np.float32)
        bvp_a = np.zeros((VW, 4), np.float32)
        bvp_a[0:HD, :] = bv_f[d0 : d0 + 256].reshape(4, HD).T

        in_maps.append(
            {
                "hT": hT_f,
                "hTq": hTq_f,
                "Wk": pack_w(Wk_f, 128),
                "Wq": pack_w(Wq_f, 128),
                "Wv": pack_w(Wv_f, 256),
                "bkq": bkq_v,
                "bvp": bvp_a,
                "pen": pens[b],
            }
        )
    return in_maps


def _unshard(plan, results):
    out = np.empty((B, S, H), np.float32)
    for c in range(NC):
        b = c // 4
        hg = c % 4
        o = np.asarray(results[c]["out"]).astype(np.float32).reshape(4, VW, S)
        ctx = o[:, 0:HD, :] / o[:, HD : HD + 1, :]          # [4, 64, S]
        ctx = ctx.transpose(2, 0, 1).reshape(S, 256)        # [S perm, 256]
        out[b][plan["perms"][b], 256 * hg : 256 * hg + 256] = ctx
    return out


def _run(inputs, trace=False):
    z = _gate_z(
        np.asarray(inputs["hidden_states"], np.float32), inputs["Wg"], inputs["bg"]
    )
    plan = _make_plan(z)
    nc = _get_nc(plan)
    in_maps = _prep_inputs(plan, **inputs)
    res = run_bass_kernel_spmd(nc, in_maps, core_ids=list(range(NC)), trace=trace)
    return _unshard(plan, res.results), res


def kernel(**inputs) -> np.ndarray:
    out, _ = _run(inputs)
    return out
